# revision 1
# baseline (speedup 1.0000x reference)
"""Trainium2 Bass kernel for nn_CausalFreqMixer (causal depthwise long-conv mixer).

Math: p = x @ W + b -> [v, g1, g2] ; for each stage: v = irfft(rfft(v,4096)*Hs)[:L] * gs.

Implementation: full-DFT-as-matmul. The per-channel frequency filter is a
pointwise multiply; the DFT/IDFT along seq are channel-shared dense matrices,
so they run on the TensorEngine as [2048x2048] @ [2048x512] matmuls with the
natural [seq-partition, channel-free] layout (no transposes anywhere).

Packed-real spectrum: bins 0..2047 with Re(X[2048]) (Nyquist) packed into the
Im slot of bin 0. Forward and inverse both use the SAME two symmetric
matrices, Fc[t,k]=cos(2*pi*t*k/4096) and S0[t,k]=-sin(2*pi*t*k/4096); all
irfft scaling is folded into the host-precomputed filter spectra, and the
packed DC/Nyquist slots are handled with cheap rank-1 matmul fixups.

Sharding: 8 cores = 4 batch samples x 2 channel halves. Zero cross-core
communication; the projection weight is column-sharded to each core's
channels.
"""
import numpy as np
from contextlib import ExitStack

import concourse.bass as bass
import concourse.bacc as bacc
import concourse.tile as tile
import concourse.mybir as mybir
from concourse.bass_utils import run_bass_kernel_spmd

try:
    import ml_dtypes
    _NP_BF16 = ml_dtypes.bfloat16
except ImportError:  # pragma: no cover
    _NP_BF16 = None

# ---- problem constants (hardcoded per contract) ----
B, L, D = 4, 2048, 1024
NFFT = 2 * L
ORDER = 2
N_CORES = 8
C = D // (N_CORES // B)      # 512 channels per core
NT = L // 128                # 16 seq tiles
ND = D // 128                # 8 contraction tiles for the projection
E = (ORDER + 1) * C          # 1536 projected columns per core

# ---- tuning knobs (defaults used by kernel()) ----
MM_MODE = "f32r"             # "f32" | "f32r" | "bf16"
REPEAT = 1

_ALU = mybir.AluOpType


def _mdt(mode):
    if mode == "bf16":
        return mybir.dt.bfloat16
    if mode == "f16":
        return mybir.dt.float16
    if mode == "f32r":
        return mybir.dt.float32r
    return mybir.dt.float32


def _np_mdt(mode):
    if mode == "bf16":
        return _NP_BF16
    if mode == "f16":
        return np.float16
    return np.float32


def _emit(nc, mode, repeat):
    mdt = _mdt(mode)
    f32 = mybir.dt.float32

    def mm(out, lhsT, rhs, start, stop):
        nc.tensor.matmul(out, lhsT, rhs, start=start, stop=stop)

    xt = nc.dram_tensor("xt", [D, L], mdt, kind="ExternalInput").ap()
    w = nc.dram_tensor("w", [D, E], mdt, kind="ExternalInput").ap()
    bias = nc.dram_tensor("bias", [1, E], f32, kind="ExternalInput").ap()
    fct = nc.dram_tensor("fct", [NT, NT, 128, 128], mdt, kind="ExternalInput").ap()
    s0t = nc.dram_tensor("s0t", [NT, NT, 128, 128], mdt, kind="ExternalInput").ap()
    altc = nc.dram_tensor("altc", [L], mdt, kind="ExternalInput").ap()
    altr = nc.dram_tensor("altr", [1, 128], mdt, kind="ExternalInput").ap()
    ha = nc.dram_tensor("ha", [ORDER, L, C], f32, kind="ExternalInput").ap()
    hb = nc.dram_tensor("hb", [ORDER, L, C], f32, kind="ExternalInput").ap()
    hd0 = nc.dram_tensor("hd0", [1, ORDER * C], f32, kind="ExternalInput").ap()
    out_d = nc.dram_tensor("out", [L, C], f32, kind="ExternalOutput").ap()

    xt_r = xt.rearrange("(kd p) l -> p kd l", p=128)
    w_r = w.rearrange("(kd p) e -> p kd e", p=128)
    altc_r = altc.rearrange("(j p) -> p j", p=128)

    with tile.TileContext(nc) as tc:
        with ExitStack() as ctx:
            consts = ctx.enter_context(tc.tile_pool(name="consts", bufs=1))
            bigs = ctx.enter_context(tc.tile_pool(name="bigs", bufs=1))
            dram = ctx.enter_context(tc.tile_pool(name="dram", bufs=1, space="DRAM"))
            psum = ctx.enter_context(tc.tile_pool(name="psum", bufs=6, space="PSUM"))
            tpool = ctx.enter_context(tc.tile_pool(name="tpool", bufs=6))

            altc_s = consts.tile([128, NT], mdt)
            nc.sync.dma_start(out=altc_s, in_=altc_r)
            altr_s = consts.tile([1, 128], mdt)
            nc.sync.dma_start(out=altr_s, in_=altr)
            hd0_s = consts.tile([1, ORDER * C], f32)
            nc.sync.dma_start(out=hd0_s, in_=hd0)
            bias_s = consts.tile([128, E], f32)
            nc.sync.dma_start(out=bias_s, in_=bias.partition_broadcast(128))

            v_buf = bigs.tile([128, NT, C], mdt)
            yr_buf = bigs.tile([128, NT, C], mdt)
            yi_buf = bigs.tile([128, NT, C], mdt)

            p_scr = dram.tile([ORDER, L, C], f32)

            for _rep in range(repeat):
                # ---------------- projection ----------------
                with tc.tile_pool(name="wproj", bufs=1) as wproj, \
                        tc.tile_pool(name="xtp", bufs=2) as xtp:
                    w_s = wproj.tile([128, ND, E], mdt)
                    for kd in range(ND):
                        nc.sync.dma_start(out=w_s[:, kd, :], in_=w_r[:, kd, :])
                    for lt in range(NT):
                        xts = xtp.tile([128, ND, 128], mdt, tag="xts")
                        nc.sync.dma_start(
                            out=xts, in_=xt_r[:, :, lt * 128:(lt + 1) * 128]
                        )
                        for ch in range(ORDER + 1):
                            ps = psum.tile([128, C], f32, tag="ps")
                            for kd in range(ND):
                                mm(ps, xts[:, kd, :], w_s[:, kd, ch * C:(ch + 1) * C],
                                   start=(kd == 0), stop=(kd == ND - 1))
                            bsl = bias_s[:, ch * C:(ch + 1) * C]
                            if ch == 0:
                                nc.vector.scalar_tensor_tensor(
                                    out=v_buf[:, lt, :], in0=ps, scalar=1.0,
                                    in1=bsl, op0=_ALU.mult, op1=_ALU.add)
                            else:
                                g = tpool.tile([128, C], f32, tag="t")
                                nc.vector.scalar_tensor_tensor(
                                    out=g, in0=ps, scalar=1.0,
                                    in1=bsl, op0=_ALU.mult, op1=_ALU.add)
                                nc.sync.dma_start(
                                    out=p_scr[ch - 1, lt * 128:(lt + 1) * 128, :],
                                    in_=g)

                # ---------------- stages ----------------
                wstage_cm = tc.tile_pool(name="wstage", bufs=3)
                wstage = wstage_cm.__enter__()
                xpool_cm = tc.tile_pool(name="xpool", bufs=3)
                xpool = xpool_cm.__enter__()
                hpool_cm = tc.tile_pool(name="hpool", bufs=3)
                hpool = hpool_cm.__enter__()
                for st in range(ORDER):
                    # forward DFT + pointwise, per bin-tile m
                    for m in range(NT):
                        wf = wstage.tile([128, NT, 128], mdt, tag="wf")
                        nc.sync.dma_start(out=wf, in_=fct[m].transpose([1, 0, 2]))
                        ws = wstage.tile([128, NT, 128], mdt, tag="ws")
                        nc.sync.dma_start(out=ws, in_=s0t[m].transpose([1, 0, 2]))

                        psxr = psum.tile([128, C], f32, tag="ps")
                        for j in range(NT):
                            mm(psxr, wf[:, j, :], v_buf[:, j, :],
                               start=(j == 0), stop=(j == NT - 1))
                        psxi = psum.tile([128, C], f32, tag="ps")
                        for j in range(NT - 1):
                            mm(psxi, ws[:, j, :], v_buf[:, j, :],
                               start=(j == 0), stop=False)
                        if m == 0:
                            # packed-Nyquist row: Xi[0] += sum_t (-1)^t v[t]
                            for j in range(NT):
                                mm(psxi[0:1, :], altc_s[:, j:j + 1], v_buf[:, j, :],
                                   start=False, stop=False)
                        mm(psxi, ws[:, NT - 1, :], v_buf[:, NT - 1, :],
                           start=False, stop=True)

                        xr = xpool.tile([128, C], f32, tag="x")
                        nc.scalar.copy(out=xr, in_=psxr)
                        xi = xpool.tile([128, C], f32, tag="x")
                        nc.scalar.copy(out=xi, in_=psxi)

                        hat = hpool.tile([128, C], f32, tag="ha")
                        nc.sync.dma_start(out=hat, in_=ha[st, m * 128:(m + 1) * 128, :])
                        hbt = hpool.tile([128, C], f32, tag="hb")
                        nc.sync.dma_start(out=hbt, in_=hb[st, m * 128:(m + 1) * 128, :])

                        t1 = tpool.tile([128, C], f32, tag="t")
                        nc.vector.tensor_mul(t1, xr, hat)
                        t2 = tpool.tile([128, C], f32, tag="t")
                        nc.vector.tensor_mul(t2, xi, hbt)
                        nc.vector.tensor_sub(yr_buf[:, m, :], t1, t2)
                        t3 = tpool.tile([128, C], f32, tag="t")
                        nc.vector.tensor_mul(t3, xr, hbt)
                        t4 = tpool.tile([128, C], f32, tag="t")
                        nc.vector.tensor_mul(t4, xi, hat)
                        nc.vector.tensor_add(yi_buf[:, m, :], t3, t4)
                        if m == 0:
                            # packed slot: Yi[0] = Xi[0] * ReH[Nyq] * 1/N
                            nc.vector.tensor_mul(
                                yi_buf[0:1, 0, :], xi[0:1, :],
                                hd0_s[0:1, st * C:(st + 1) * C])

                    # inverse DFT + gate, per time-tile mt
                    for mt in range(NT):
                        wfi = wstage.tile([128, NT, 128], mdt, tag="wf")
                        nc.sync.dma_start(out=wfi, in_=fct[mt].transpose([1, 0, 2]))
                        wsi = wstage.tile([128, NT, 128], mdt, tag="ws")
                        nc.sync.dma_start(out=wsi, in_=s0t[mt].transpose([1, 0, 2]))

                        psc = psum.tile([128, C], f32, tag="ps")
                        for j in range(NT):
                            mm(psc, wfi[:, j, :], yr_buf[:, j, :],
                               start=(j == 0), stop=False)
                        for j in range(NT):
                            mm(psc, wsi[:, j, :], yi_buf[:, j, :],
                               start=False, stop=False)
                        # packed slot contribution: conv[t] += (-1)^t * Yi[0]
                        mm(psc, altr_s[0:1, :],
                           yi_buf[0:1, 0, :], start=False, stop=True)

                        gin = tpool.tile([128, C], f32, tag="t")
                        nc.sync.dma_start(
                            out=gin, in_=p_scr[st, mt * 128:(mt + 1) * 128, :])
                        if st < ORDER - 1:
                            nc.vector.scalar_tensor_tensor(
                                out=v_buf[:, mt, :], in0=psc, scalar=1.0,
                                in1=gin, op0=_ALU.mult, op1=_ALU.mult)
                        else:
                            og = tpool.tile([128, C], f32, tag="t")
                            nc.vector.scalar_tensor_tensor(
                                out=og, in0=psc, scalar=1.0,
                                in1=gin, op0=_ALU.mult, op1=_ALU.mult)
                            nc.sync.dma_start(
                                out=out_d[mt * 128:(mt + 1) * 128, :], in_=og)
                hpool_cm.__exit__(None, None, None)
                xpool_cm.__exit__(None, None, None)
                wstage_cm.__exit__(None, None, None)


_PROGRAMS = {}


def build_program(mode=None, repeat=None):
    mode = MM_MODE if mode is None else mode
    repeat = REPEAT if repeat is None else repeat
    key = (mode, repeat)
    if key not in _PROGRAMS:
        nc = bacc.Bacc("TRN2", target_bir_lowering=False, debug=False,
                       enable_asserts=False, num_devices=N_CORES)
        _emit(nc, mode, repeat)
        nc.compile()
        _PROGRAMS[key] = nc
    return _PROGRAMS[key]


_TABLES = {}


def host_tables(mode=None):
    """Shared DFT matrices, pre-tiled as [m, j, p, k] = M[128j+p, 128m+k]."""
    mode = MM_MODE if mode is None else mode
    if mode not in _TABLES:
        npdt = _np_mdt(mode)
        t = np.arange(L, dtype=np.float64)
        ang = (2.0 * np.pi / NFFT) * np.outer(t, t)
        fc = np.cos(ang)
        s0 = -np.sin(ang)

        def tile4(mat):
            return np.ascontiguousarray(
                mat.reshape(NT, 128, NT, 128).transpose(2, 0, 1, 3).astype(npdt))

        alt = ((-1.0) ** np.arange(L))
        _TABLES[mode] = {
            "fct": tile4(fc),
            "s0t": tile4(s0),
            "altc": alt.astype(npdt),
            "altr": np.ascontiguousarray(alt[:128].reshape(1, 128).astype(npdt)),
        }
    return _TABLES[mode]


def filter_spectra(filter_time):
    """Packed, scale-folded filter spectra per stage: (A, B, d0) with
    Yr = Xr*A - Xi*B ; Yi = Xr*B + Xi*A except Yi[0] = Xi[0]*d0."""
    out = []
    for stg in range(ORDER):
        h = np.asarray(filter_time[stg, 0], dtype=np.float64)   # [L, D]
        H = np.fft.rfft(h, n=NFFT, axis=0)                       # [L+1, D]
        s = np.full((L, 1), 2.0 / NFFT)
        s[0, 0] = 1.0 / NFFT
        A = (H[:L].real * s).astype(np.float32)
        Bm = (H[:L].imag * s).astype(np.float32)
        Bm[0, :] = 0.0
        d0 = (H[L].real / NFFT).astype(np.float32)               # [D]
        out.append((A, Bm, d0))
    return out


def make_in_maps(x, proj_w, proj_b, filter_time, mode=None):
    mode = MM_MODE if mode is None else mode
    npdt = _np_mdt(mode)
    tables = host_tables(mode)
    specs = filter_spectra(filter_time)
    in_maps = []
    for core in range(N_CORES):
        b, half = divmod(core, N_CORES // B)
        c0 = half * C
        cols = np.concatenate(
            [np.arange(s * D + c0, s * D + c0 + C) for s in range(ORDER + 1)])
        ha = np.stack([specs[stg][0][:, c0:c0 + C] for stg in range(ORDER)])
        hb = np.stack([specs[stg][1][:, c0:c0 + C] for stg in range(ORDER)])
        hd0 = np.concatenate(
            [specs[stg][2][c0:c0 + C] for stg in range(ORDER)]).reshape(1, ORDER * C)
        in_maps.append({
            "xt": np.ascontiguousarray(np.asarray(x[b]).T.astype(npdt)),
            "w": np.ascontiguousarray(np.asarray(proj_w)[:, cols].astype(npdt)),
            "bias": np.ascontiguousarray(
                np.asarray(proj_b)[cols].astype(np.float32).reshape(1, E)),
            "fct": tables["fct"],
            "s0t": tables["s0t"],
            "altc": tables["altc"],
            "altr": tables["altr"],
            "ha": np.ascontiguousarray(ha.astype(np.float32)),
            "hb": np.ascontiguousarray(hb.astype(np.float32)),
            "hd0": np.ascontiguousarray(hd0.astype(np.float32)),
        })
    return in_maps


def gather_out(results):
    out = np.zeros((B, L, D), dtype=np.float32)
    for core in range(N_CORES):
        b, half = divmod(core, N_CORES // B)
        c0 = half * C
        out[b, :, c0:c0 + C] = results[core]["out"]
    return out


def kernel(x, proj_w, proj_b, filter_time):
    # Pull inputs to host numpy up front: device->host transfers must happen
    # BEFORE the bass NEFF executes (exec can leave the PJRT device in a
    # state where later transfers of pre-existing device arrays fail).
    x = np.asarray(x)
    proj_w = np.asarray(proj_w)
    proj_b = np.asarray(proj_b)
    filter_time = np.asarray(filter_time)
    nc = build_program()
    in_maps = make_in_maps(x, proj_w, proj_b, filter_time)
    res = run_bass_kernel_spmd(nc, in_maps, list(range(N_CORES)))
    return gather_out(res.results)



# revision 21
# speedup vs baseline: 719.7676x; 719.7676x over previous
"""Trainium2 Bass kernel for nn_CausalFreqMixer (causal depthwise long-conv mixer).

Math: p = x @ W + b -> [v, g1, g2] ; for each stage: v = causalconv(v, h_s) * g_s.

Implementation: overlap-save block FFT. The 2048-long causal conv splits into
4 blocks of 512; each stage does per-block packed-real DFT-1024 as matmuls
(shared [512x512] cos/sin matrices, seq-partition x channel-free layout), a
frequency-domain multiply-accumulate over the 16 (block, lag) product pairs
on the vector/gpsimd engines in fp16, and a per-block inverse DFT as matmuls.
This cuts tensor-engine work ~2.6x and HBM traffic ~7x vs a full 4096-point
DFT: the DFT matrices shrink from 2x16MB (streamed 4x) to 2x0.5MB, SBUF
resident.

Packed-real spectrum: bins 0..511 with Re(X[512]) (Nyquist) packed into the
Im slot of bin 0. Forward and inverse both use the SAME two symmetric
matrices Fc[t,k]=cos(2*pi*t*k/1024), S0[t,k]=-sin(2*pi*t*k/1024); irfft
scaling is folded into the host-precomputed per-lag filter spectra. The
overlap-save tail (time shift by 512) is a per-bin (-1)^k sign, applied as a
per-partition scalar in the fused multiply-accumulate.

Precision: projection in f32r; everything downstream (DFT matmuls, spectra,
pointwise MAC, gates) in fp16 (validated ~1e-3 rel err vs 2e-2 tolerance).

Sharding: 8 cores = 4 batch samples x 2 channel halves. Zero cross-core
communication; the projection weight is column-sharded to each core's
channels.
"""
import numpy as np
from contextlib import ExitStack

import concourse.bass as bass
import concourse.bacc as bacc
import concourse.tile as tile
import concourse.mybir as mybir
from concourse.bass_utils import run_bass_kernel_spmd

# ---- problem constants (hardcoded per contract) ----
B, L, D = 4, 2048, 1024
ORDER = 2
N_CORES = 8
C = D // (N_CORES // B)      # 512 channels per core
ND = D // 128                # 8 contraction tiles for the projection
E = (ORDER + 1) * C          # 1536 projected columns per core

BETA = 512                   # conv block size
NB = L // BETA               # 4 blocks
NFFT = 2 * BETA              # 1024
M = BETA                     # packed bins
MT = M // 128                # 4 bin/time tiles per block
NT = L // 128                # 16 seq tiles of v_buf

REPEAT = 1
MM_MODE = "f32r"             # projection matmul dtype (stages are fp16)
PW_POOL_EVERY = 5            # every k-th product pair runs on gpsimd (Pool)
NB1 = NB - 1                 # tail products only use lags d < NB-1

_ALU = mybir.AluOpType


def _pairs(i):
    """(src block jj, lag d, is_tail) products for output block i."""
    out = [(i - d, d, False) for d in range(NB) if 0 <= i - d]
    out += [(i - 1 - d, d, True) for d in range(NB) if 0 <= i - 1 - d]
    return out


def _emit(nc, mode, repeat):
    f32 = mybir.dt.float32
    f16 = mybir.dt.float16
    pdt = mybir.dt.float32r if mode == "f32r" else mybir.dt.float32

    def mm(out, lhsT, rhs, start, stop):
        nc.tensor.matmul(out, lhsT, rhs, start=start, stop=stop)

    xt = nc.dram_tensor("xt", [D, L], pdt, kind="ExternalInput").ap()
    w = nc.dram_tensor("w", [D, E], pdt, kind="ExternalInput").ap()
    bias = nc.dram_tensor("bias", [1, E], f32, kind="ExternalInput").ap()
    fct = nc.dram_tensor("fct", [MT, MT, 128, 128], f16, kind="ExternalInput").ap()
    s0t = nc.dram_tensor("s0t", [MT, MT, 128, 128], f16, kind="ExternalInput").ap()
    altc = nc.dram_tensor("altc", [M], f16, kind="ExternalInput").ap()
    altr = nc.dram_tensor("altr", [1, 128], f16, kind="ExternalInput").ap()
    ha = nc.dram_tensor("ha", [ORDER, NB, M, C], f16, kind="ExternalInput").ap()
    hb = nc.dram_tensor("hb", [ORDER, NB, M, C], f16, kind="ExternalInput").ap()
    ha2 = nc.dram_tensor("ha2", [ORDER, NB1, M, C], f16, kind="ExternalInput").ap()
    hb2 = nc.dram_tensor("hb2", [ORDER, NB1, M, C], f16, kind="ExternalInput").ap()
    hd0 = nc.dram_tensor("hd0", [1, ORDER * NB * C], f16, kind="ExternalInput").ap()
    out_d = nc.dram_tensor("out", [L, C], f32, kind="ExternalOutput").ap()

    xt_r = xt.rearrange("(kd p) l -> p kd l", p=128)
    w_r = w.rearrange("(kd p) e -> p kd e", p=128)
    altc_r = altc.rearrange("(j p) -> p j", p=128)
    ha_r = ha.rearrange("s d (kt p) c -> s d p kt c", p=128)
    hb_r = hb.rearrange("s d (kt p) c -> s d p kt c", p=128)
    ha2_r = ha2.rearrange("s d (kt p) c -> s d p kt c", p=128)
    hb2_r = hb2.rearrange("s d (kt p) c -> s d p kt c", p=128)

    with tile.TileContext(nc) as tc:
        with ExitStack() as ctx:
            consts = ctx.enter_context(tc.tile_pool(name="consts", bufs=1))
            bigs = ctx.enter_context(tc.tile_pool(name="bigs", bufs=1))
            hpool = ctx.enter_context(tc.tile_pool(name="hpool", bufs=1))
            psum = ctx.enter_context(tc.tile_pool(name="psum", bufs=6, space="PSUM"))

            fc_s = consts.tile([128, MT, MT, 128], f16)
            nc.sync.dma_start(out=fc_s, in_=fct.transpose([2, 0, 1, 3]))
            s0_s = consts.tile([128, MT, MT, 128], f16)
            nc.sync.dma_start(out=s0_s, in_=s0t.transpose([2, 0, 1, 3]))
            altc_s = consts.tile([128, MT], f16)
            nc.sync.dma_start(out=altc_s, in_=altc_r)
            altr_s = consts.tile([1, 128], f16)
            nc.sync.dma_start(out=altr_s, in_=altr)
            hd0_s = consts.tile([1, ORDER * NB * C], f16)
            nc.sync.dma_start(out=hd0_s, in_=hd0)
            bias_s = consts.tile([128, E], f32)
            nc.sync.dma_start(out=bias_s, in_=bias.partition_broadcast(128))

            v_buf = bigs.tile([128, NT, C], f16)
            g_buf = bigs.tile([128, NT, C], f16)
            vr_s = bigs.tile([128, NB, MT, C], f16)
            vi_s = bigs.tile([128, NB, MT, C], f16)

            ha_s = hpool.tile([128, NB, MT, C], f16)
            hb_s = hpool.tile([128, NB, MT, C], f16)
            ha2_s = hpool.tile([128, NB1, MT, C], f16)
            hb2_s = hpool.tile([128, NB1, MT, C], f16)

            dram = ctx.enter_context(tc.tile_pool(name="dram", bufs=1, space="DRAM"))
            g2_scr = dram.tile([L, C], f16)

            for _rep in range(repeat):
                # ---------------- projection ----------------
                with tc.tile_pool(name="wproj", bufs=1) as wproj, \
                        tc.tile_pool(name="xtp", bufs=2) as xtp, \
                        tc.tile_pool(name="spool2", bufs=3) as spool2:
                    w_s = wproj.tile([128, ND, E], pdt)
                    for kd in range(ND):
                        nc.sync.dma_start(out=w_s[:, kd, :], in_=w_r[:, kd, :])
                    for lt in range(NT):
                        xts = xtp.tile([128, ND, 128], pdt, tag="xts")
                        nc.sync.dma_start(
                            out=xts, in_=xt_r[:, :, lt * 128:(lt + 1) * 128]
                        )
                        for ch in range(ORDER + 1):
                            ps = psum.tile([128, C], f32, tag="ps")
                            for kd in range(ND):
                                mm(ps, xts[:, kd, :], w_s[:, kd, ch * C:(ch + 1) * C],
                                   start=(kd == 0), stop=(kd == ND - 1))
                            bsl = bias_s[:, ch * C:(ch + 1) * C]
                            if ch < 2:
                                dst = v_buf[:, lt, :] if ch == 0 else g_buf[:, lt, :]
                                nc.vector.scalar_tensor_tensor(
                                    out=dst, in0=ps, scalar=1.0,
                                    in1=bsl, op0=_ALU.mult, op1=_ALU.add)
                            else:
                                g2t = spool2.tile([128, C], f16, tag="g2")
                                nc.vector.scalar_tensor_tensor(
                                    out=g2t, in0=ps, scalar=1.0,
                                    in1=bsl, op0=_ALU.mult, op1=_ALU.add)
                                nc.sync.dma_start(
                                    out=g2_scr[lt * 128:(lt + 1) * 128, :], in_=g2t)

                # ---------------- stages ----------------
                stage_cm = ExitStack()
                rpool = stage_cm.enter_context(tc.tile_pool(name="rpool", bufs=2))
                tpd = stage_cm.enter_context(tc.tile_pool(name="tpd", bufs=3))
                tpp = stage_cm.enter_context(tc.tile_pool(name="tpp", bufs=3))
                spool = stage_cm.enter_context(tc.tile_pool(name="spool", bufs=1))
                for st in range(ORDER):
                    for d in range(NB):
                        nc.sync.dma_start(out=ha_s[:, d, :, :], in_=ha_r[st, d])
                        nc.sync.dma_start(out=hb_s[:, d, :, :], in_=hb_r[st, d])
                    for d in range(NB1):
                        nc.sync.dma_start(out=ha2_s[:, d, :, :], in_=ha2_r[st, d])
                        nc.sync.dma_start(out=hb2_s[:, d, :, :], in_=hb2_r[st, d])
                    if st == 1:
                        # stage-2 gate back from its DRAM round-trip
                        for lt in range(NT):
                            nc.sync.dma_start(
                                out=g_buf[:, lt, :],
                                in_=g2_scr[lt * 128:(lt + 1) * 128, :])

                    # forward DFT of the 4 blocks
                    for j in range(NB):
                        for m in range(MT):
                            psxr = psum.tile([128, C], f32, tag="ps")
                            for jj in range(MT):
                                mm(psxr, fc_s[:, m, jj, :], v_buf[:, NB * j + jj, :],
                                   start=(jj == 0), stop=(jj == MT - 1))
                            psxi = psum.tile([128, C], f32, tag="ps")
                            for jj in range(MT - 1):
                                mm(psxi, s0_s[:, m, jj, :], v_buf[:, NB * j + jj, :],
                                   start=(jj == 0), stop=False)
                            if m == 0:
                                # packed-Nyquist row: Xi[0] = sum_t (-1)^t v[t]
                                for jj in range(MT):
                                    mm(psxi[0:1, :], altc_s[:, jj:jj + 1],
                                       v_buf[:, NB * j + jj, :],
                                       start=False, stop=False)
                            mm(psxi, s0_s[:, m, MT - 1, :], v_buf[:, NB * j + MT - 1, :],
                               start=False, stop=True)
                            nc.scalar.copy(out=vr_s[:, j, m, :], in_=psxr)
                            nc.scalar.copy(out=vi_s[:, j, m, :], in_=psxi)

                    # frequency-domain MAC + inverse DFT per output block
                    for i in range(NB):
                        prods = _pairs(i)
                        rlist = []   # accumulators per engine: (eng, rr, ri)
                        nyq = None
                        for (pi, (jj, d, tail)) in enumerate(prods):
                            # one product per output block on gpsimd (Pool
                            # runs tensor ops at ~0.42 efficiency, so it only
                            # earns ~20% of the pointwise work).
                            use_pool = (pi == 1) and len(prods) > 2
                            eng = nc.gpsimd if use_pool else nc.vector
                            tp = tpp if use_pool else tpd
                            ent = next((e for e in rlist if e[0] is eng), None)
                            first = ent is None
                            if first:
                                rr = rpool.tile([128, MT, C], f16,
                                                tag=f"rr{len(rlist)}")
                                ri = rpool.tile([128, MT, C], f16,
                                                tag=f"ri{len(rlist)}")
                                rlist.append((eng, rr, ri))
                            else:
                                _, rr, ri = ent
                            # tail products use the (-1)^k-folded tables
                            A = (ha2_s if tail else ha_s)[:, d, :, :]
                            Bm = (hb2_s if tail else hb_s)[:, d, :, :]
                            Vr = vr_s[:, jj, :, :]
                            Vi = vi_s[:, jj, :, :]
                            t1 = tp.tile([128, MT, C], f16, tag="t")
                            eng.tensor_mul(t1, Vr, A)
                            t2 = tp.tile([128, MT, C], f16, tag="t")
                            eng.tensor_mul(t2, Vi, Bm)
                            if first:
                                eng.tensor_sub(rr, t1, t2)
                            else:
                                eng.tensor_add(rr, rr, t1)
                                eng.tensor_sub(rr, rr, t2)
                            t3 = tp.tile([128, MT, C], f16, tag="t")
                            eng.tensor_mul(t3, Vr, Bm)
                            t4 = tp.tile([128, MT, C], f16, tag="t")
                            eng.tensor_mul(t4, Vi, A)
                            if first:
                                eng.tensor_add(ri, t3, t4)
                            else:
                                eng.tensor_add(ri, ri, t3)
                                eng.tensor_add(ri, ri, t4)
                            # packed-Nyquist slot accumulation (sign always +1)
                            d0sl = hd0_s[0:1, (st * NB + d) * C:(st * NB + d + 1) * C]
                            if nyq is None:
                                nyq = spool.tile([1, C], f32, tag="nyq")
                                nc.vector.tensor_mul(nyq, vi_s[0:1, jj, 0, :], d0sl)
                            else:
                                tn = spool.tile([1, C], f32, tag="tn")
                                nc.vector.tensor_mul(tn, vi_s[0:1, jj, 0, :], d0sl)
                                nc.vector.tensor_add(nyq, nyq, tn)
                        # combine per-engine accumulators
                        eng0, rr0, ri0 = rlist[0]
                        for (_, rrx, rix) in rlist[1:]:
                            nc.vector.tensor_add(rr0, rr0, rrx)
                            nc.vector.tensor_add(ri0, ri0, rix)
                        nc.scalar.copy(out=ri0[0:1, 0, :], in_=nyq)

                        # inverse DFT + gate
                        for mt in range(MT):
                            psc = psum.tile([128, C], f32, tag="ps")
                            for jk in range(MT):
                                mm(psc, fc_s[:, mt, jk, :], rr0[:, jk, :],
                                   start=(jk == 0), stop=False)
                            for jk in range(MT):
                                mm(psc, s0_s[:, mt, jk, :], ri0[:, jk, :],
                                   start=False, stop=False)
                            mm(psc, altr_s[0:1, :], ri0[0:1, 0, :],
                               start=False, stop=True)
                            gsl = g_buf[:, NB * i + mt, :]
                            if st < ORDER - 1:
                                nc.vector.scalar_tensor_tensor(
                                    out=v_buf[:, NB * i + mt, :], in0=psc,
                                    scalar=1.0, in1=gsl,
                                    op0=_ALU.mult, op1=_ALU.mult)
                            else:
                                og = spool.tile([128, C], f32, tag="og")
                                nc.vector.scalar_tensor_tensor(
                                    out=og, in0=psc, scalar=1.0, in1=gsl,
                                    op0=_ALU.mult, op1=_ALU.mult)
                                nc.sync.dma_start(
                                    out=out_d[(NB * i + mt) * 128:
                                              (NB * i + mt + 1) * 128, :], in_=og)
                stage_cm.close()


_PROGRAMS = {}


def build_program(mode=None, repeat=None):
    mode = MM_MODE if mode is None else mode
    repeat = REPEAT if repeat is None else repeat
    key = (mode, repeat)
    if key not in _PROGRAMS:
        nc = bacc.Bacc("TRN2", target_bir_lowering=False, debug=False,
                       enable_asserts=False, num_devices=N_CORES)
        _emit(nc, mode, repeat)
        nc.compile()
        _PROGRAMS[key] = nc
    return _PROGRAMS[key]


_TABLES = None


def host_tables():
    """Shared DFT matrices, pre-tiled as [m, j, p, k] = Mat[128j+p, 128m+k]."""
    global _TABLES
    if _TABLES is None:
        t = np.arange(M, dtype=np.float64)
        ang = (2.0 * np.pi / NFFT) * np.outer(t, t)
        fc = np.cos(ang)
        s0 = -np.sin(ang)

        def tile4(mat):
            return np.ascontiguousarray(
                mat.reshape(MT, 128, MT, 128).transpose(2, 0, 1, 3)
                .astype(np.float16))

        alt = ((-1.0) ** np.arange(M))
        _TABLES = {
            "fct": tile4(fc),
            "s0t": tile4(s0),
            "altc": alt.astype(np.float16),
            "altr": np.ascontiguousarray(
                alt[:128].reshape(1, 128).astype(np.float16)),
        }
    return _TABLES


def filter_spectra(filter_time):
    """Per (stage, lag-block) packed scale-folded spectra: A, B [M, D], d0 [D],
    plus (-1)^k-folded copies A2, B2 for the overlap-save tail products."""
    A = np.zeros((ORDER, NB, M, D), dtype=np.float16)
    Bm = np.zeros((ORDER, NB, M, D), dtype=np.float16)
    A2 = np.zeros((ORDER, NB1, M, D), dtype=np.float16)
    B2 = np.zeros((ORDER, NB1, M, D), dtype=np.float16)
    d0 = np.zeros((ORDER, NB, D), dtype=np.float16)
    s = np.full((M, 1), 2.0 / NFFT)
    s[0, 0] = 1.0 / NFFT
    sgn = ((-1.0) ** np.arange(M))[:, None]
    for stg in range(ORDER):
        for d in range(NB):
            h = np.asarray(filter_time[stg, 0, d * BETA:(d + 1) * BETA],
                           dtype=np.float64)                  # [BETA, D]
            H = np.fft.rfft(h, n=NFFT, axis=0)                # [M+1, D]
            a = H[:M].real * s
            b = H[:M].imag * s
            b[0, :] = 0.0
            A[stg, d] = a.astype(np.float16)
            Bm[stg, d] = b.astype(np.float16)
            if d < NB1:
                A2[stg, d] = (a * sgn).astype(np.float16)
                B2[stg, d] = (b * sgn).astype(np.float16)
            d0[stg, d] = (H[M].real / NFFT).astype(np.float16)
    return A, Bm, A2, B2, d0


def make_in_maps(x, proj_w, proj_b, filter_time, mode=None):
    tables = host_tables()
    A, Bm, A2, B2, d0 = filter_spectra(filter_time)
    in_maps = []
    for core in range(N_CORES):
        b, half = divmod(core, N_CORES // B)
        c0 = half * C
        cols = np.concatenate(
            [np.arange(s * D + c0, s * D + c0 + C) for s in range(ORDER + 1)])
        in_maps.append({
            "xt": np.ascontiguousarray(np.asarray(x[b]).T.astype(np.float32)),
            "w": np.ascontiguousarray(np.asarray(proj_w)[:, cols].astype(np.float32)),
            "bias": np.ascontiguousarray(
                np.asarray(proj_b)[cols].astype(np.float32).reshape(1, E)),
            "fct": tables["fct"],
            "s0t": tables["s0t"],
            "altc": tables["altc"],
            "altr": tables["altr"],
            "ha": np.ascontiguousarray(A[:, :, :, c0:c0 + C]),
            "hb": np.ascontiguousarray(Bm[:, :, :, c0:c0 + C]),
            "ha2": np.ascontiguousarray(A2[:, :, :, c0:c0 + C]),
            "hb2": np.ascontiguousarray(B2[:, :, :, c0:c0 + C]),
            "hd0": np.ascontiguousarray(
                d0[:, :, c0:c0 + C].reshape(1, ORDER * NB * C)),
        })
    return in_maps


def gather_out(results):
    out = np.zeros((B, L, D), dtype=np.float32)
    for core in range(N_CORES):
        b, half = divmod(core, N_CORES // B)
        c0 = half * C
        out[b, :, c0:c0 + C] = results[core]["out"]
    return out


def kernel(x, proj_w, proj_b, filter_time):
    # Pull inputs to host numpy up front: device->host transfers must happen
    # BEFORE the bass NEFF executes (exec can leave the PJRT device in a
    # state where later transfers of pre-existing device arrays fail).
    x = np.asarray(x)
    proj_w = np.asarray(proj_w)
    proj_b = np.asarray(proj_b)
    filter_time = np.asarray(filter_time)
    nc = build_program()
    in_maps = make_in_maps(x, proj_w, proj_b, filter_time)
    res = run_bass_kernel_spmd(nc, in_maps, list(range(N_CORES)))
    return gather_out(res.results)


# revision 31
# speedup vs baseline: 939.0330x; 1.3046x over previous
"""Trainium2 Bass kernel for nn_CausalFreqMixer (causal depthwise long-conv mixer).

Math: p = x @ W + b -> [v, g1, g2] ; for each stage: v = causalconv(v, h_s) * g_s.

Implementation: overlap-save block FFT. The 2048-long causal conv splits into
4 blocks of 512; each stage does per-block packed-real DFT-1024 as matmuls
(shared [512x512] cos/sin matrices, seq-partition x channel-free layout), a
frequency-domain multiply-accumulate over the 16 (block, lag) product pairs
on the vector/gpsimd engines in fp16, and a per-block inverse DFT as matmuls.
This cuts tensor-engine work ~2.6x and HBM traffic ~7x vs a full 4096-point
DFT: the DFT matrices shrink from 2x16MB (streamed 4x) to 2x0.5MB, SBUF
resident.

Packed-real spectrum: bins 0..511 with Re(X[512]) (Nyquist) packed into the
Im slot of bin 0. Forward and inverse both use the SAME two symmetric
matrices Fc[t,k]=cos(2*pi*t*k/1024), S0[t,k]=-sin(2*pi*t*k/1024); irfft
scaling is folded into the host-precomputed per-lag filter spectra. The
overlap-save tail (time shift by 512) is a per-bin (-1)^k sign, applied as a
per-partition scalar in the fused multiply-accumulate.

Precision: projection in f32r; everything downstream (DFT matmuls, spectra,
pointwise MAC, gates) in fp16 (validated ~1e-3 rel err vs 2e-2 tolerance).

Sharding: 8 cores = 4 batch samples x 2 channel halves. Zero cross-core
communication; the projection weight is column-sharded to each core's
channels.
"""
import numpy as np
from contextlib import ExitStack

import concourse.bass as bass
import concourse.bacc as bacc
import concourse.tile as tile
import concourse.mybir as mybir
from concourse.bass_utils import run_bass_kernel_spmd

# ---- problem constants (hardcoded per contract) ----
B, L, D = 4, 2048, 1024
ORDER = 2
N_CORES = 8
C = D // (N_CORES // B)      # 512 channels per core
ND = D // 128                # 8 contraction tiles for the projection
E = (ORDER + 1) * C          # 1536 projected columns per core

BETA = 512                   # conv block size
NB = L // BETA               # 4 blocks
NFFT = 2 * BETA              # 1024
M = BETA                     # packed bins
MT = M // 128                # 4 bin/time tiles per block
NT = L // 128                # 16 seq tiles of v_buf

REPEAT = 1
MM_MODE = "f32r"             # projection matmul dtype (stages are fp16)
PW_POOL_EVERY = 5            # every k-th product pair runs on gpsimd (Pool)
NB1 = NB - 1                 # tail products only use lags d < NB-1

_ALU = mybir.AluOpType


def _pairs(i):
    """(src block jj, lag d, is_tail) products for output block i."""
    out = [(i - d, d, False) for d in range(NB) if 0 <= i - d]
    out += [(i - 1 - d, d, True) for d in range(NB) if 0 <= i - 1 - d]
    return out


def _emit(nc, mode, repeat):
    f32 = mybir.dt.float32
    f16 = mybir.dt.float16
    pdt = mybir.dt.float32r if mode == "f32r" else mybir.dt.float32

    def mm(out, lhsT, rhs, start, stop):
        nc.tensor.matmul(out, lhsT, rhs, start=start, stop=stop)

    xt = nc.dram_tensor("xt", [D, L], pdt, kind="ExternalInput").ap()
    w = nc.dram_tensor("w", [D, E], pdt, kind="ExternalInput").ap()
    bias = nc.dram_tensor("bias", [1, E], f32, kind="ExternalInput").ap()
    fct = nc.dram_tensor("fct", [MT, MT, 128, 128], f16, kind="ExternalInput").ap()
    s0t = nc.dram_tensor("s0t", [MT, MT, 128, 128], f16, kind="ExternalInput").ap()
    altc = nc.dram_tensor("altc", [M], f16, kind="ExternalInput").ap()
    altr = nc.dram_tensor("altr", [1, 128], f16, kind="ExternalInput").ap()
    ha = nc.dram_tensor("ha", [ORDER, NB, M, C], f16, kind="ExternalInput").ap()
    hb = nc.dram_tensor("hb", [ORDER, NB, M, C], f16, kind="ExternalInput").ap()
    fc2t = nc.dram_tensor("fc2t", [MT, MT, 128, 128], f16, kind="ExternalInput").ap()
    s02t = nc.dram_tensor("s02t", [MT, MT, 128, 128], f16, kind="ExternalInput").ap()
    hd0 = nc.dram_tensor("hd0", [1, ORDER * NB * C], f16, kind="ExternalInput").ap()
    out_d = nc.dram_tensor("out", [L, C], f32, kind="ExternalOutput").ap()

    xt_r = xt.rearrange("(kd p) l -> p kd l", p=128)
    w_r = w.rearrange("(kd p) e -> p kd e", p=128)
    altc_r = altc.rearrange("(j p) -> p j", p=128)
    ha_r = ha.rearrange("s d (kt p) c -> s d p kt c", p=128)
    hb_r = hb.rearrange("s d (kt p) c -> s d p kt c", p=128)

    with tile.TileContext(nc) as tc:
        with ExitStack() as ctx:
            consts = ctx.enter_context(tc.tile_pool(name="consts", bufs=1))
            bigs = ctx.enter_context(tc.tile_pool(name="bigs", bufs=1))
            hpool = ctx.enter_context(tc.tile_pool(name="hpool", bufs=1))
            psum = ctx.enter_context(tc.tile_pool(name="psum", bufs=6, space="PSUM"))

            fc_s = consts.tile([128, MT, MT, 128], f16)
            nc.sync.dma_start(out=fc_s, in_=fct.transpose([2, 0, 1, 3]))
            s0_s = consts.tile([128, MT, MT, 128], f16)
            nc.sync.dma_start(out=s0_s, in_=s0t.transpose([2, 0, 1, 3]))
            fc2_s = consts.tile([128, MT, MT, 128], f16)
            nc.sync.dma_start(out=fc2_s, in_=fc2t.transpose([2, 0, 1, 3]))
            s02_s = consts.tile([128, MT, MT, 128], f16)
            nc.sync.dma_start(out=s02_s, in_=s02t.transpose([2, 0, 1, 3]))
            altc_s = consts.tile([128, MT], f16)
            nc.sync.dma_start(out=altc_s, in_=altc_r)
            altr_s = consts.tile([1, 128], f16)
            nc.sync.dma_start(out=altr_s, in_=altr)
            hd0_s = consts.tile([1, ORDER * NB * C], f16)
            nc.sync.dma_start(out=hd0_s, in_=hd0)
            bias_s = consts.tile([128, E], f32)
            nc.sync.dma_start(out=bias_s, in_=bias.partition_broadcast(128))

            v_buf = bigs.tile([128, NT, C], f16)
            g_buf = bigs.tile([128, NT, C], f16)
            vr_s = bigs.tile([128, NB, MT, C], f16)
            vi_s = bigs.tile([128, NB, MT, C], f16)

            ha_s = hpool.tile([128, NB, MT, C], f16)
            hb_s = hpool.tile([128, NB, MT, C], f16)

            dram = ctx.enter_context(tc.tile_pool(name="dram", bufs=1, space="DRAM"))
            g2_scr = dram.tile([L, C], f16)

            for _rep in range(repeat):
                # ---------------- projection ----------------
                with tc.tile_pool(name="wproj", bufs=1) as wproj, \
                        tc.tile_pool(name="xtp", bufs=2) as xtp, \
                        tc.tile_pool(name="spool2", bufs=3) as spool2:
                    w_s = wproj.tile([128, ND, E], pdt)
                    for kd in range(ND):
                        nc.sync.dma_start(out=w_s[:, kd, :], in_=w_r[:, kd, :])
                    for lt in range(NT):
                        xts = xtp.tile([128, ND, 128], pdt, tag="xts")
                        nc.sync.dma_start(
                            out=xts, in_=xt_r[:, :, lt * 128:(lt + 1) * 128]
                        )
                        for ch in range(ORDER + 1):
                            ps = psum.tile([128, C], f32, tag="ps")
                            for kd in range(ND):
                                mm(ps, xts[:, kd, :], w_s[:, kd, ch * C:(ch + 1) * C],
                                   start=(kd == 0), stop=(kd == ND - 1))
                            bsl = bias_s[:, ch * C:(ch + 1) * C]
                            if ch < 2:
                                dst = v_buf[:, lt, :] if ch == 0 else g_buf[:, lt, :]
                                nc.vector.scalar_tensor_tensor(
                                    out=dst, in0=ps, scalar=1.0,
                                    in1=bsl, op0=_ALU.mult, op1=_ALU.add)
                            else:
                                g2t = spool2.tile([128, C], f16, tag="g2")
                                nc.vector.scalar_tensor_tensor(
                                    out=g2t, in0=ps, scalar=1.0,
                                    in1=bsl, op0=_ALU.mult, op1=_ALU.add)
                                nc.sync.dma_start(
                                    out=g2_scr[lt * 128:(lt + 1) * 128, :], in_=g2t)

                # ---------------- stages ----------------
                stage_cm = ExitStack()
                rpool = stage_cm.enter_context(tc.tile_pool(name="rpool", bufs=2))
                tpd = stage_cm.enter_context(tc.tile_pool(name="tpd", bufs=3))
                tpp = stage_cm.enter_context(tc.tile_pool(name="tpp", bufs=3))
                spool = stage_cm.enter_context(tc.tile_pool(name="spool", bufs=1))
                for st in range(ORDER):
                    for d in range(NB):
                        nc.sync.dma_start(out=ha_s[:, d, :, :], in_=ha_r[st, d])
                        nc.sync.dma_start(out=hb_s[:, d, :, :], in_=hb_r[st, d])
                    if st == 1:
                        # stage-2 gate back from its DRAM round-trip
                        for lt in range(NT):
                            nc.sync.dma_start(
                                out=g_buf[:, lt, :],
                                in_=g2_scr[lt * 128:(lt + 1) * 128, :])

                    # forward DFT of the 4 blocks
                    for j in range(NB):
                        for m in range(MT):
                            psxr = psum.tile([128, C], f32, tag="ps")
                            for jj in range(MT):
                                mm(psxr, fc_s[:, m, jj, :], v_buf[:, NB * j + jj, :],
                                   start=(jj == 0), stop=(jj == MT - 1))
                            psxi = psum.tile([128, C], f32, tag="ps")
                            for jj in range(MT - 1):
                                mm(psxi, s0_s[:, m, jj, :], v_buf[:, NB * j + jj, :],
                                   start=(jj == 0), stop=False)
                            if m == 0:
                                # packed-Nyquist row: Xi[0] = sum_t (-1)^t v[t]
                                for jj in range(MT):
                                    mm(psxi[0:1, :], altc_s[:, jj:jj + 1],
                                       v_buf[:, NB * j + jj, :],
                                       start=False, stop=False)
                            mm(psxi, s0_s[:, m, MT - 1, :], v_buf[:, NB * j + MT - 1, :],
                               start=False, stop=True)
                            nc.scalar.copy(out=vr_s[:, j, m, :], in_=psxr)
                            nc.scalar.copy(out=vi_s[:, j, m, :], in_=psxi)

                    # frequency-domain MAC + inverse DFT per output block.
                    # Overlap-save tail spectrum Q_i == P_{i-1}, so only head
                    # products are computed; the previous block's accumulator
                    # enters the inverse through (-1)^k-folded DFT matrices.
                    rrPrev = riPrev = None
                    for i in range(NB):
                        prods = [(i - d, d) for d in range(i + 1)]
                        rlist = []   # accumulators per engine: (eng, rr, ri)
                        nyq = None
                        for (pi, (jj, d)) in enumerate(prods):
                            # one product per output block on gpsimd (Pool
                            # runs tensor ops at ~0.42 efficiency, so it only
                            # earns ~20% of the pointwise work).
                            use_pool = (pi == 1) and len(prods) > 2
                            eng = nc.gpsimd if use_pool else nc.vector
                            tp = tpp if use_pool else tpd
                            ent = next((e for e in rlist if e[0] is eng), None)
                            first = ent is None
                            if first:
                                rr = rpool.tile([128, MT, C], f16,
                                                tag=f"rr{len(rlist)}")
                                ri = rpool.tile([128, MT, C], f16,
                                                tag=f"ri{len(rlist)}")
                                rlist.append((eng, rr, ri))
                            else:
                                _, rr, ri = ent
                            A = ha_s[:, d, :, :]
                            Bm = hb_s[:, d, :, :]
                            Vr = vr_s[:, jj, :, :]
                            Vi = vi_s[:, jj, :, :]
                            t1 = tp.tile([128, MT, C], f16, tag="t")
                            eng.tensor_mul(t1, Vr, A)
                            t2 = tp.tile([128, MT, C], f16, tag="t")
                            eng.tensor_mul(t2, Vi, Bm)
                            if first:
                                eng.tensor_sub(rr, t1, t2)
                            else:
                                eng.tensor_add(rr, rr, t1)
                                eng.tensor_sub(rr, rr, t2)
                            t3 = tp.tile([128, MT, C], f16, tag="t")
                            eng.tensor_mul(t3, Vr, Bm)
                            t4 = tp.tile([128, MT, C], f16, tag="t")
                            eng.tensor_mul(t4, Vi, A)
                            if first:
                                eng.tensor_add(ri, t3, t4)
                            else:
                                eng.tensor_add(ri, ri, t3)
                                eng.tensor_add(ri, ri, t4)
                            # packed-Nyquist slot accumulation (sign always +1)
                            d0sl = hd0_s[0:1, (st * NB + d) * C:(st * NB + d + 1) * C]
                            if nyq is None:
                                nyq = spool.tile([1, C], f32, tag="nyq")
                                nc.vector.tensor_mul(nyq, vi_s[0:1, jj, 0, :], d0sl)
                            else:
                                tn = spool.tile([1, C], f32, tag="tn")
                                nc.vector.tensor_mul(tn, vi_s[0:1, jj, 0, :], d0sl)
                                nc.vector.tensor_add(nyq, nyq, tn)
                        # combine per-engine accumulators
                        eng0, rr0, ri0 = rlist[0]
                        for (_, rrx, rix) in rlist[1:]:
                            nc.vector.tensor_add(rr0, rr0, rrx)
                            nc.vector.tensor_add(ri0, ri0, rix)
                        nc.scalar.copy(out=ri0[0:1, 0, :], in_=nyq)

                        # inverse DFT + gate (tail via sign-folded matrices)
                        for mt in range(MT):
                            psc = psum.tile([128, C], f32, tag="ps")
                            for jk in range(MT):
                                mm(psc, fc_s[:, mt, jk, :], rr0[:, jk, :],
                                   start=(jk == 0), stop=False)
                            for jk in range(MT):
                                mm(psc, s0_s[:, mt, jk, :], ri0[:, jk, :],
                                   start=False, stop=False)
                            if rrPrev is not None:
                                for jk in range(MT):
                                    mm(psc, fc2_s[:, mt, jk, :], rrPrev[:, jk, :],
                                       start=False, stop=False)
                                for jk in range(MT):
                                    mm(psc, s02_s[:, mt, jk, :], riPrev[:, jk, :],
                                       start=False, stop=False)
                                mm(psc, altr_s[0:1, :], riPrev[0:1, 0, :],
                                   start=False, stop=False)
                            mm(psc, altr_s[0:1, :], ri0[0:1, 0, :],
                               start=False, stop=True)
                            gsl = g_buf[:, NB * i + mt, :]
                            if st < ORDER - 1:
                                nc.vector.scalar_tensor_tensor(
                                    out=v_buf[:, NB * i + mt, :], in0=psc,
                                    scalar=1.0, in1=gsl,
                                    op0=_ALU.mult, op1=_ALU.mult)
                            else:
                                og = spool.tile([128, C], f32, tag="og")
                                nc.vector.scalar_tensor_tensor(
                                    out=og, in0=psc, scalar=1.0, in1=gsl,
                                    op0=_ALU.mult, op1=_ALU.mult)
                                nc.sync.dma_start(
                                    out=out_d[(NB * i + mt) * 128:
                                              (NB * i + mt + 1) * 128, :], in_=og)
                        rrPrev, riPrev = rr0, ri0
                stage_cm.close()


_PROGRAMS = {}


def build_program(mode=None, repeat=None):
    mode = MM_MODE if mode is None else mode
    repeat = REPEAT if repeat is None else repeat
    key = (mode, repeat)
    if key not in _PROGRAMS:
        nc = bacc.Bacc("TRN2", target_bir_lowering=False, debug=False,
                       enable_asserts=False, num_devices=N_CORES)
        _emit(nc, mode, repeat)
        nc.compile()
        _PROGRAMS[key] = nc
    return _PROGRAMS[key]


_TABLES = None


def host_tables():
    """Shared DFT matrices, pre-tiled as [m, j, p, k] = Mat[128j+p, 128m+k]."""
    global _TABLES
    if _TABLES is None:
        t = np.arange(M, dtype=np.float64)
        ang = (2.0 * np.pi / NFFT) * np.outer(t, t)
        fc = np.cos(ang)
        s0 = -np.sin(ang)

        def tile4(mat):
            return np.ascontiguousarray(
                mat.reshape(MT, 128, MT, 128).transpose(2, 0, 1, 3)
                .astype(np.float16))

        alt = ((-1.0) ** np.arange(M))
        _TABLES = {
            "fct": tile4(fc),
            "s0t": tile4(s0),
            "fc2t": tile4(alt[:, None] * fc),
            "s02t": tile4(alt[:, None] * s0),
            "altc": alt.astype(np.float16),
            "altr": np.ascontiguousarray(
                alt[:128].reshape(1, 128).astype(np.float16)),
        }
    return _TABLES


def filter_spectra(filter_time):
    """Per (stage, lag-block) packed scale-folded spectra: A, B [M, D], d0 [D],
    plus (-1)^k-folded copies A2, B2 for the overlap-save tail products."""
    A = np.zeros((ORDER, NB, M, D), dtype=np.float16)
    Bm = np.zeros((ORDER, NB, M, D), dtype=np.float16)
    A2 = np.zeros((ORDER, NB1, M, D), dtype=np.float16)
    B2 = np.zeros((ORDER, NB1, M, D), dtype=np.float16)
    d0 = np.zeros((ORDER, NB, D), dtype=np.float16)
    s = np.full((M, 1), 2.0 / NFFT)
    s[0, 0] = 1.0 / NFFT
    sgn = ((-1.0) ** np.arange(M))[:, None]
    for stg in range(ORDER):
        for d in range(NB):
            h = np.asarray(filter_time[stg, 0, d * BETA:(d + 1) * BETA],
                           dtype=np.float64)                  # [BETA, D]
            H = np.fft.rfft(h, n=NFFT, axis=0)                # [M+1, D]
            a = H[:M].real * s
            b = H[:M].imag * s
            b[0, :] = 0.0
            A[stg, d] = a.astype(np.float16)
            Bm[stg, d] = b.astype(np.float16)
            if d < NB1:
                A2[stg, d] = (a * sgn).astype(np.float16)
                B2[stg, d] = (b * sgn).astype(np.float16)
            d0[stg, d] = (H[M].real / NFFT).astype(np.float16)
    return A, Bm, A2, B2, d0


def make_in_maps(x, proj_w, proj_b, filter_time, mode=None):
    tables = host_tables()
    A, Bm, A2, B2, d0 = filter_spectra(filter_time)
    in_maps = []
    for core in range(N_CORES):
        b, half = divmod(core, N_CORES // B)
        c0 = half * C
        cols = np.concatenate(
            [np.arange(s * D + c0, s * D + c0 + C) for s in range(ORDER + 1)])
        in_maps.append({
            "xt": np.ascontiguousarray(np.asarray(x[b]).T.astype(np.float32)),
            "w": np.ascontiguousarray(np.asarray(proj_w)[:, cols].astype(np.float32)),
            "bias": np.ascontiguousarray(
                np.asarray(proj_b)[cols].astype(np.float32).reshape(1, E)),
            "fct": tables["fct"],
            "s0t": tables["s0t"],
            "altc": tables["altc"],
            "altr": tables["altr"],
            "fc2t": tables["fc2t"],
            "s02t": tables["s02t"],
            "ha": np.ascontiguousarray(A[:, :, :, c0:c0 + C]),
            "hb": np.ascontiguousarray(Bm[:, :, :, c0:c0 + C]),
            "hd0": np.ascontiguousarray(
                d0[:, :, c0:c0 + C].reshape(1, ORDER * NB * C)),
        })
    return in_maps


def gather_out(results):
    out = np.zeros((B, L, D), dtype=np.float32)
    for core in range(N_CORES):
        b, half = divmod(core, N_CORES // B)
        c0 = half * C
        out[b, :, c0:c0 + C] = results[core]["out"]
    return out


def kernel(x, proj_w, proj_b, filter_time):
    # Pull inputs to host numpy up front: device->host transfers must happen
    # BEFORE the bass NEFF executes (exec can leave the PJRT device in a
    # state where later transfers of pre-existing device arrays fail).
    x = np.asarray(x)
    proj_w = np.asarray(proj_w)
    proj_b = np.asarray(proj_b)
    filter_time = np.asarray(filter_time)
    nc = build_program()
    in_maps = make_in_maps(x, proj_w, proj_b, filter_time)
    res = run_bass_kernel_spmd(nc, in_maps, list(range(N_CORES)))
    return gather_out(res.results)
